# revision 1
# baseline (speedup 1.0000x reference)
"""DynamicConv2D Trainium2 kernel (8-core SPMD, data-parallel over batch).

Per sample: GAP -> MLP -> softmax routing over K=4 kernel banks, weight-space
aggregation, then a 3x3 SAME conv with the per-sample aggregated kernel.

Device strategy (per core, 4 samples, fully per-sample pipelined):
  - Host packs x into a width-padded, channel-duplicated bf16 layout
    [SP=128*130, 128] so one DMA-xbar-transpose load yields xT
    [128 part = (c | c dup), spatial'] in SBUF with zero columns at the
    image edges (SAME padding in w) and zero halo in SBUF (SAME in h).
  - Pooled mean via DVE/ACT free-dim reduction over xT.
  - Tiny routing MLP on PE (fp32) + softmax (DVE/ACT) -> pi [1, 4].
  - pi broadcast to all partitions (gpsimd), kernel bank aggregated on DVE
    with scalar_tensor_tensor FMA chains -> per-sample W_agg bf16 stationary.
  - Conv as shifted matmuls accumulating in PSUM: out[f, p] tiles, f on
    partitions. SBUF partitions 64:128 hold x shifted one padded image row
    up (the row above), so one K=128 matmul computes taps (dy=0, dx) and
    (dy=-1, dx) at once; the dy=+1 taps are K=64 matmuls at row base 0.
    Col groups (0/64) of the PE array run the two half-image tiles A/B
    concurrently. All xbar-transpose DMA jobs are kept small (<=40 xbar
    tiles) and per-ring homogeneous: bigger jobs post more than the 16
    semaphore increments Tile's cumulative waits assume, and mixing
    transpose/plain jobs on one HWDGE ring forces xbar-mode serialization.
  - ACT drains PSUM (+per-f bias) to bf16 yT, DMA-xbar transposes back to
    [p, f], bf16 store to DRAM; host strips width pads and upcasts to fp32.
"""

import numpy as np
import ml_dtypes

BF16 = ml_dtypes.bfloat16

B, H, W, C, F = 32, 128, 128, 64, 64
KK, HID = 4, 16
TEMP = 30.0
NCORES, BPC = 8, 4
WP = W + 2          # padded width (zero col at w'=0 and w'=129)
SP = H * WP         # 16640 padded spatial per sample
PAD = 256           # SBUF halo each side; xbar output offsets must be 128-aligned
NT = 416            # matmul moving-dim tile (PSUM bank: <=512 fp32)
HALF = SP // 2      # 8320, image halves A (h<64) / B (h>=64)
TPH = HALF // NT    # 20 tiles per half
NCHUNK = HALF // 128  # 65 output xbar chunks per sample
NSLOT = 6             # 3 paired-tap slots (K=128) + 3 single-tap slots (K=64)
OCH = 5               # 128-col blocks per output DMA-transpose job (<=64 xbar tiles)

_CACHE = {}


def _build_program(dbg=False, reps=1):
    import concourse.bacc as bacc
    import concourse.mybir as mybir
    import concourse.tile as tile

    f32 = mybir.dt.float32
    bf16 = mybir.dt.bfloat16
    AX = mybir.AxisListType.X
    ALU = mybir.AluOpType
    ACTF = mybir.ActivationFunctionType

    nc = bacc.Bacc("TRN2", target_bir_lowering=False, debug=False)

    x2_d = nc.dram_tensor("x2", [BPC, SP, 128], bf16, kind="ExternalInput")
    wk_d = nc.dram_tensor("wk", [128, KK * NSLOT * F], f32,
                          kind="ExternalInput")
    w1_d = nc.dram_tensor("w1", [C, HID], f32, kind="ExternalInput")
    b1_d = nc.dram_tensor("b1", [HID, 1], f32, kind="ExternalInput")
    w2_d = nc.dram_tensor("w2", [HID, KK], f32, kind="ExternalInput")
    b2_d = nc.dram_tensor("b2", [1, KK], f32, kind="ExternalInput")
    bkt_d = nc.dram_tensor("bkt", [128, KK], f32, kind="ExternalInput")
    yp_d = nc.dram_tensor("ypad", [BPC, SP, F], bf16, kind="ExternalOutput")
    if dbg:
        dxt_d = nc.dram_tensor("dxt", [BPC, 128, 512], bf16,
                               kind="ExternalOutput")
        dpool_d = nc.dram_tensor("dpool", [BPC, C, 1], f32,
                                 kind="ExternalOutput")
        dpib_d = nc.dram_tensor("dpib", [BPC, 128, KK], f32,
                                kind="ExternalOutput")
        dwg_d = nc.dram_tensor("dwg", [BPC, 128, NSLOT * F], bf16,
                               kind="ExternalOutput")
        dyt_d = nc.dram_tensor("dyt", [BPC, 128, 512], bf16,
                               kind="ExternalOutput")

    with tile.TileContext(nc) as tc:
        from contextlib import ExitStack
        with ExitStack() as ctx:
            cst = ctx.enter_context(tc.tile_pool(name="cst", bufs=1))
            xtp = ctx.enter_context(tc.tile_pool(name="xtp", bufs=3))
            ytp = ctx.enter_context(tc.tile_pool(name="ytp", bufs=2))
            ysp = ctx.enter_context(tc.tile_pool(name="ysp", bufs=2))
            wgp = ctx.enter_context(tc.tile_pool(name="wgp", bufs=2))
            smp = ctx.enter_context(tc.tile_pool(name="smp", bufs=2))
            psp = ctx.enter_context(tc.tile_pool(name="psp", bufs=6, space="PSUM"))
            psr = ctx.enter_context(tc.tile_pool(name="psr", bufs=1, space="PSUM"))

            # ---- constants ----
            wk_t = cst.tile([128, KK * NSLOT * F], f32)
            nc.sync.dma_start(wk_t[:], wk_d.ap())
            w1_t = cst.tile([C, HID], f32)
            nc.sync.dma_start(w1_t[:], w1_d.ap())
            b1_t = cst.tile([HID, 1], f32)
            nc.sync.dma_start(b1_t[:], b1_d.ap())
            w2_t = cst.tile([HID, KK], f32)
            nc.sync.dma_start(w2_t[:], w2_d.ap())
            b2_t = cst.tile([1, KK], f32)
            nc.sync.dma_start(b2_t[:], b2_d.ap())
            bkt_t = cst.tile([128, KK], f32)
            nc.sync.dma_start(bkt_t[:], bkt_d.ap())
            bagg_t = cst.tile([128, BPC], f32)
            trash = cst.tile([C, 2114], bf16)

            for _rep in range(reps):
              for b in range(BPC):
                # ---- load + transpose x ----
                xt = xtp.tile([128, PAD + SP + PAD], bf16, tag="xt")
                nc.gpsimd.memset(xt[:, 0:PAD], 0.0)
                nc.gpsimd.memset(xt[:, PAD + SP:PAD + SP + PAD], 0.0)
                for s in range(SP // 128):
                    nc.sync.dma_start(
                        xt[:, PAD + s * 128:PAD + (s + 1) * 128],
                        x2_d.ap()[b][s * 128:(s + 1) * 128, :],
                        transpose=True)

                # ---- pooled sum (free-dim reduce; pads are zero) ----
                pp = smp.tile([C, 6], f32, tag="pp")
                nc.vector.reduce_sum(pp[:, 0:1], xt[0:C, 0:8696], axis=AX)
                for i in range(4):
                    s0 = 8696 + i * 2114
                    nc.scalar.activation(trash[:], xt[0:C, s0:s0 + 2114],
                                         ACTF.Copy,
                                         accum_out=pp[:, 1 + i:2 + i])
                pooled = smp.tile([C, 1], f32, tag="pooled")
                nc.vector.reduce_sum(pooled[:], pp[:, 0:5], axis=AX)

                # ---- routing MLP (fp32, tiny) ----
                hps = psr.tile([HID, 1], f32, tag="hps")
                nc.tensor.matmul(hps[:], lhsT=w1_t[:], rhs=pooled[:],
                                 start=True, stop=True)
                h_t = smp.tile([HID, 1], f32, tag="h")
                nc.scalar.activation(h_t[:], hps[:], ACTF.Relu,
                                     bias=b1_t[:], scale=1.0)
                lps = psr.tile([1, KK], f32, tag="lps")
                nc.tensor.matmul(lps[:], lhsT=h_t[:], rhs=w2_t[:],
                                 start=True, stop=True)
                lg = smp.tile([1, KK], f32, tag="lg")
                nc.vector.tensor_tensor(lg[:], lps[:], b2_t[:], op=ALU.add)
                mx = smp.tile([1, 1], f32, tag="mx")
                nc.vector.reduce_max(mx[:], lg[:], axis=AX)
                ex = smp.tile([1, KK], f32, tag="ex")
                nc.vector.tensor_scalar(ex[:], lg[:], scalar1=mx[:],
                                        scalar2=None, op0=ALU.subtract)
                nc.scalar.activation(ex[:], ex[:], ACTF.Exp)
                sm = smp.tile([1, 1], f32, tag="sm")
                nc.vector.reduce_sum(sm[:], ex[:], axis=AX)
                rc = smp.tile([1, 1], f32, tag="rc")
                nc.vector.reciprocal(rc[:], sm[:])
                pi_t = smp.tile([1, KK], f32, tag="pi")
                nc.vector.tensor_scalar(pi_t[:], ex[:], scalar1=rc[:],
                                        scalar2=None, op0=ALU.mult)
                pib = smp.tile([128, KK], f32, tag="pib")
                nc.gpsimd.partition_broadcast(pib[:], pi_t[:])

                # ---- per-sample bias column: bagg[:, b] = sum_k bkT[:,k]*pi_k
                nc.vector.tensor_scalar(bagg_t[:, b:b + 1], bkt_t[:, 0:1],
                                        scalar1=pib[:, 0:1], scalar2=None,
                                        op0=ALU.mult)
                for k in range(1, KK):
                    nc.vector.scalar_tensor_tensor(
                        bagg_t[:, b:b + 1], bkt_t[:, k:k + 1],
                        pib[:, k:k + 1], bagg_t[:, b:b + 1],
                        op0=ALU.mult, op1=ALU.add)

                # ---- aggregate kernel bank: W_agg = sum_k pi_k * Wk ----
                SF = NSLOT * F
                acc = wgp.tile([128, SF], f32, tag="acc")
                nc.vector.tensor_scalar(acc[:], wk_t[:, 0:SF],
                                        scalar1=pib[:, 0:1], scalar2=None,
                                        op0=ALU.mult)
                for k in range(1, KK):
                    nc.vector.scalar_tensor_tensor(
                        acc[:], wk_t[:, k * SF:(k + 1) * SF],
                        pib[:, k:k + 1], acc[:], op0=ALU.mult, op1=ALU.add)
                wg = wgp.tile([128, SF], bf16, tag="wg")
                nc.vector.tensor_copy(wg[:], acc[:])

                # ---- conv: paired-tap K=128 + single-tap K=64 matmuls ----
                yt = ytp.tile([128, HALF], bf16, tag="yt")
                for t in range(TPH):
                    ps = psp.tile([128, NT], f32, tag="ps")
                    oA = PAD + t * NT
                    oB = oA + HALF
                    for j in range(3):       # taps (0,dx)+(-1,dx), K=128
                        off = j - 1
                        nc.tensor.matmul(
                            ps[0:64, :], lhsT=wg[:, j * F:(j + 1) * F],
                            rhs=xt[:, oA + off:oA + off + NT],
                            start=(j == 0), stop=False)
                        nc.tensor.matmul(
                            ps[64:128, :], lhsT=wg[:, j * F:(j + 1) * F],
                            rhs=xt[:, oB + off:oB + off + NT],
                            start=(j == 0), stop=False,
                            tile_position=(0, 64))
                    for j in range(3, 6):    # taps (+1,dx), K=64
                        off = WP + (j - 4)
                        nc.tensor.matmul(
                            ps[0:64, :], lhsT=wg[0:64, j * F:(j + 1) * F],
                            rhs=xt[0:64, oA + off:oA + off + NT],
                            start=False, stop=(j == 5))
                        nc.tensor.matmul(
                            ps[64:128, :], lhsT=wg[0:64, j * F:(j + 1) * F],
                            rhs=xt[0:64, oB + off:oB + off + NT],
                            start=False, stop=(j == 5),
                            tile_position=(0, 64))
                    nc.scalar.activation(yt[:, t * NT:(t + 1) * NT], ps[:],
                                         ACTF.Identity,
                                         bias=bagg_t[:, b:b + 1], scale=1.0)

                if dbg:
                    nc.sync.dma_start(dxt_d.ap()[b], xt[:, PAD:PAD + 512])
                    nc.sync.dma_start(dpool_d.ap()[b], pooled[:])
                    nc.sync.dma_start(dpib_d.ap()[b], pib[:])
                    nc.sync.dma_start(dwg_d.ap()[b], wg[:])
                    nc.sync.dma_start(dyt_d.ap()[b], yt[:, 0:512])

                # ---- transpose back to [p, f] and store bf16 ----
                ys = ysp.tile([128, NCHUNK, 128], bf16, tag="ys")
                for j0 in range(0, NCHUNK, OCH):
                    j1 = min(j0 + OCH, NCHUNK)
                    nc.scalar.dma_start(ys[:, j0:j1, :],
                                        yt[:, j0 * 128:j1 * 128],
                                        transpose=True)
                ypb = yp_d.ap()[b]
                dstA = ypb[0:HALF, :].rearrange("(j u) f -> u j f", u=128)
                dstB = ypb[HALF:SP, :].rearrange("(j u) f -> u j f", u=128)
                nc.gpsimd.dma_start(dstA, ys[:, :, 0:64])
                nc.gpsimd.dma_start(dstB, ys[:, :, 64:128])

    nc.compile()
    return nc


def _get_program():
    if "nc" not in _CACHE:
        _CACHE["nc"] = _build_program()
    return _CACHE["nc"]


def _host_pack_x(x):
    # [B, H, W, C] fp32 -> [B, SP, 128] bf16: cols 0:64 = width-padded x,
    # cols 64:128 = same, shifted one padded image row (WP) up (row above).
    xb = x.astype(BF16)
    xp = np.zeros((B, H, WP, C), dtype=BF16)
    xp[:, :, 1:W + 1, :] = xb
    flat = xp.reshape(B, SP, C)
    x2 = np.zeros((B, SP, 128), dtype=BF16)
    x2[:, :, 0:C] = flat
    x2[:, WP:SP, C:2 * C] = flat[:, 0:SP - WP]
    return np.ascontiguousarray(x2)


def _host_pack_wk(Wk):
    # [K, 3, 3, C, F] -> [128, K*NSLOT*F] fp32. Slot j in 0..2 pairs taps
    # (kh=1, kw=j) on partitions 0:64 with (kh=0, kw=j) on 64:128 (the
    # bottom x half holds the row above); slot j in 3..5 holds (kh=2,
    # kw=j-3) on partitions 0:64, zeros on 64:128.
    w = np.zeros((128, KK, NSLOT, F), dtype=np.float32)
    wt = np.transpose(Wk, (3, 0, 1, 2, 4))          # [C, K, kh, kw, F]
    for j in range(3):
        w[0:C, :, j] = wt[:, :, 1, j]
        w[C:2 * C, :, j] = wt[:, :, 0, j]
        w[0:C, :, 3 + j] = wt[:, :, 2, j]
    return np.ascontiguousarray(w.reshape(128, KK * NSLOT * F))


def kernel(x, Wk, bk, att_w1, att_b1, att_w2, att_b2):
    from concourse import bass_utils

    nc = _get_program()

    x2 = _host_pack_x(np.asarray(x))
    wk_h = _host_pack_wk(np.asarray(Wk))
    w1_h = np.ascontiguousarray((att_w1 / (H * W)).astype(np.float32))
    b1_h = np.ascontiguousarray(att_b1.reshape(HID, 1).astype(np.float32))
    w2_h = np.ascontiguousarray((att_w2 / TEMP).astype(np.float32))
    b2_h = np.ascontiguousarray((att_b2 / TEMP).reshape(1, KK)
                                .astype(np.float32))
    bkt = np.transpose(bk, (1, 0)).astype(np.float32)      # [F, K]
    bkt_h = np.ascontiguousarray(np.concatenate([bkt, bkt], axis=0))

    in_maps = []
    for c in range(NCORES):
        in_maps.append({
            "x2": x2[c * BPC:(c + 1) * BPC],
            "wk": wk_h, "w1": w1_h, "b1": b1_h,
            "w2": w2_h, "b2": b2_h, "bkt": bkt_h,
        })

    res = bass_utils.run_bass_kernel_spmd(nc, in_maps,
                                          core_ids=list(range(NCORES)))

    y = np.empty((B, H, W, F), dtype=np.float32)
    for c in range(NCORES):
        yp = res.results[c]["ypad"].reshape(BPC, H, WP, F)
        y[c * BPC:(c + 1) * BPC] = yp[:, :, 1:W + 1, :].astype(np.float32)
    return y



# revision 2
# speedup vs baseline: 4.3250x; 4.3250x over previous
"""DynamicConv2D Trainium2 kernel (8-core SPMD, data-parallel over batch).

Per sample: GAP -> MLP -> softmax routing over K=4 kernel banks, weight-space
aggregation, then a 3x3 SAME conv with the per-sample aggregated kernel.

Device strategy (per core, 4 samples, fully per-sample pipelined):
  - Host packs x into a TRANSPOSED width-padded, channel-duplicated bf16
    layout [128, SP] per sample (rows 0:64 = channels at spatial s, rows
    64:128 = same data shifted one padded image row down, so row 64+c at
    position s holds x[c, s-WP]). One plain contiguous DMA per sample
    loads it -- no xbar-transpose DMAs anywhere (they serialized the SP
    engine in the previous design).
  - Pooled mean via one DVE free-dim reduction over xt rows 0:64.
  - Tiny routing MLP on PE (fp32) + softmax (DVE/ACT) -> pi [1, 4].
  - pi broadcast to all partitions (gpsimd), kernel bank aggregated on DVE
    with scalar_tensor_tensor FMA chains -> per-sample W_agg bf16.
  - Conv as shifted matmuls accumulating in PSUM: out[f, p] tiles, f on
    partitions. SBUF partitions 64:128 hold x shifted one padded image row
    up (the row above), so one K=128 matmul computes taps (dy=0, dx) and
    (dy=-1, dx) at once; the dy=+1 taps are K=64 matmuls at row base 0.
    Col groups (0/64) of the PE array run the two half-image tiles A/B
    concurrently.
  - PSUM drains (+per-f bias) alternate between ACT and DVE into bf16 yT
    [f-partitions, spatial]; one plain DMA stores yT per sample; host
    un-transposes and strips width pads, upcasts to fp32.
"""

import numpy as np
import ml_dtypes

BF16 = ml_dtypes.bfloat16

B, H, W, C, F = 32, 128, 128, 64, 64
KK, HID = 4, 16
TEMP = 30.0
NCORES, BPC = 8, 4
WP = W + 2          # padded width (zero col at w'=0 and w'=129)
SP = H * WP         # 16640 padded spatial per sample
PADL = 384          # SBUF zero halo before the image
PADR = 384          # SBUF zero halo after (taps read up to +2*WP+1 = 261)
NT = 416            # matmul moving-dim tile (PSUM bank: <=512 fp32)
HALF = SP // 2      # 8320, image halves A (h<64) / B (h>=64)
TPH = HALF // NT    # 20 tiles per half
NSLOT = 6           # 3 paired-tap slots (K=128) + 3 single-tap slots (K=64)

_CACHE = {}


def _build_program(dbg=False, reps=1):
    import concourse.bacc as bacc
    import concourse.mybir as mybir
    import concourse.tile as tile

    f32 = mybir.dt.float32
    bf16 = mybir.dt.bfloat16
    AX = mybir.AxisListType.X
    ALU = mybir.AluOpType
    ACTF = mybir.ActivationFunctionType

    nc = bacc.Bacc("TRN2", target_bir_lowering=False, debug=False)

    x2_d = nc.dram_tensor("x2", [BPC, 128, SP], bf16, kind="ExternalInput")
    wk_d = nc.dram_tensor("wk", [128, KK * NSLOT * F], f32,
                          kind="ExternalInput")
    w1_d = nc.dram_tensor("w1", [C, HID], f32, kind="ExternalInput")
    b1_d = nc.dram_tensor("b1", [HID, 1], f32, kind="ExternalInput")
    w2_d = nc.dram_tensor("w2", [HID, KK], f32, kind="ExternalInput")
    b2_d = nc.dram_tensor("b2", [1, KK], f32, kind="ExternalInput")
    bkt_d = nc.dram_tensor("bkt", [128, KK], f32, kind="ExternalInput")
    yp_d = nc.dram_tensor("ypad", [BPC, 128, HALF], bf16,
                          kind="ExternalOutput")
    if dbg:
        dxt_d = nc.dram_tensor("dxt", [BPC, 128, 512], bf16,
                               kind="ExternalOutput")
        dpool_d = nc.dram_tensor("dpool", [BPC, C, 1], f32,
                                 kind="ExternalOutput")
        dpib_d = nc.dram_tensor("dpib", [BPC, 128, KK], f32,
                                kind="ExternalOutput")
        dwg_d = nc.dram_tensor("dwg", [BPC, 128, NSLOT * F], bf16,
                               kind="ExternalOutput")

    with tile.TileContext(nc) as tc:
        from contextlib import ExitStack
        with ExitStack() as ctx:
            cst = ctx.enter_context(tc.tile_pool(name="cst", bufs=1))
            xtp = ctx.enter_context(tc.tile_pool(name="xtp", bufs=3))
            ytp = ctx.enter_context(tc.tile_pool(name="ytp", bufs=2))
            wgp = ctx.enter_context(tc.tile_pool(name="wgp", bufs=2))
            smp = ctx.enter_context(tc.tile_pool(name="smp", bufs=2))
            psp = ctx.enter_context(tc.tile_pool(name="psp", bufs=6, space="PSUM"))
            psr = ctx.enter_context(tc.tile_pool(name="psr", bufs=1, space="PSUM"))

            # ---- constants ----
            wk_t = cst.tile([128, KK * NSLOT * F], f32)
            nc.sync.dma_start(wk_t[:], wk_d.ap())
            w1_t = cst.tile([C, HID], f32)
            nc.sync.dma_start(w1_t[:], w1_d.ap())
            b1_t = cst.tile([HID, 1], f32)
            nc.sync.dma_start(b1_t[:], b1_d.ap())
            w2_t = cst.tile([HID, KK], f32)
            nc.sync.dma_start(w2_t[:], w2_d.ap())
            b2_t = cst.tile([1, KK], f32)
            nc.sync.dma_start(b2_t[:], b2_d.ap())
            bkt_t = cst.tile([128, KK], f32)
            nc.sync.dma_start(bkt_t[:], bkt_d.ap())
            bagg_t = cst.tile([128, BPC], f32)

            for _rep in range(reps):
              for b in range(BPC):
                # ---- load x (plain DMA, already transposed on host) ----
                xt = xtp.tile([128, PADL + SP + PADR], bf16, tag="xt")
                nc.gpsimd.memset(xt[:, 0:PADL], 0.0)
                nc.gpsimd.memset(xt[:, PADL + SP:PADL + SP + PADR], 0.0)
                nc.sync.dma_start(xt[:, PADL:PADL + SP], x2_d.ap()[b])

                # ---- pooled sum (free-dim reduce; pads are zero) ----
                pooled = smp.tile([C, 1], f32, tag="pooled")
                nc.vector.reduce_sum(pooled[:], xt[0:C, PADL:PADL + SP],
                                     axis=AX)

                # ---- routing MLP (fp32, tiny) ----
                hps = psr.tile([HID, 1], f32, tag="hps")
                nc.tensor.matmul(hps[:], lhsT=w1_t[:], rhs=pooled[:],
                                 start=True, stop=True)
                h_t = smp.tile([HID, 1], f32, tag="h")
                nc.scalar.activation(h_t[:], hps[:], ACTF.Relu,
                                     bias=b1_t[:], scale=1.0)
                lps = psr.tile([1, KK], f32, tag="lps")
                nc.tensor.matmul(lps[:], lhsT=h_t[:], rhs=w2_t[:],
                                 start=True, stop=True)
                lg = smp.tile([1, KK], f32, tag="lg")
                nc.vector.tensor_tensor(lg[:], lps[:], b2_t[:], op=ALU.add)
                mx = smp.tile([1, 1], f32, tag="mx")
                nc.vector.reduce_max(mx[:], lg[:], axis=AX)
                ex = smp.tile([1, KK], f32, tag="ex")
                nc.vector.tensor_scalar(ex[:], lg[:], scalar1=mx[:],
                                        scalar2=None, op0=ALU.subtract)
                nc.scalar.activation(ex[:], ex[:], ACTF.Exp)
                sm = smp.tile([1, 1], f32, tag="sm")
                nc.vector.reduce_sum(sm[:], ex[:], axis=AX)
                rc = smp.tile([1, 1], f32, tag="rc")
                nc.vector.reciprocal(rc[:], sm[:])
                pi_t = smp.tile([1, KK], f32, tag="pi")
                nc.vector.tensor_scalar(pi_t[:], ex[:], scalar1=rc[:],
                                        scalar2=None, op0=ALU.mult)
                pib = smp.tile([128, KK], f32, tag="pib")
                nc.gpsimd.partition_broadcast(pib[:], pi_t[:])

                # ---- per-sample bias column: bagg[:, b] = sum_k bkT[:,k]*pi_k
                nc.vector.tensor_scalar(bagg_t[:, b:b + 1], bkt_t[:, 0:1],
                                        scalar1=pib[:, 0:1], scalar2=None,
                                        op0=ALU.mult)
                for k in range(1, KK):
                    nc.vector.scalar_tensor_tensor(
                        bagg_t[:, b:b + 1], bkt_t[:, k:k + 1],
                        pib[:, k:k + 1], bagg_t[:, b:b + 1],
                        op0=ALU.mult, op1=ALU.add)

                # ---- aggregate kernel bank: W_agg = sum_k pi_k * Wk ----
                SF = NSLOT * F
                acc = wgp.tile([128, SF], f32, tag="acc")
                nc.vector.tensor_scalar(acc[:], wk_t[:, 0:SF],
                                        scalar1=pib[:, 0:1], scalar2=None,
                                        op0=ALU.mult)
                for k in range(1, KK):
                    nc.vector.scalar_tensor_tensor(
                        acc[:], wk_t[:, k * SF:(k + 1) * SF],
                        pib[:, k:k + 1], acc[:], op0=ALU.mult, op1=ALU.add)
                wg = wgp.tile([128, SF], bf16, tag="wg")
                nc.vector.tensor_copy(wg[:], acc[:])

                # ---- conv: paired-tap K=128 + single-tap K=64 matmuls ----
                yt = ytp.tile([128, HALF], bf16, tag="yt")
                for t in range(TPH):
                    ps = psp.tile([128, NT], f32, tag="ps")
                    oA = PADL + t * NT
                    oB = oA + HALF
                    for j in range(3):       # taps (0,dx)+(-1,dx), K=128
                        off = j - 1
                        nc.tensor.matmul(
                            ps[0:64, :], lhsT=wg[:, j * F:(j + 1) * F],
                            rhs=xt[:, oA + off:oA + off + NT],
                            start=(j == 0), stop=False)
                        nc.tensor.matmul(
                            ps[64:128, :], lhsT=wg[:, j * F:(j + 1) * F],
                            rhs=xt[:, oB + off:oB + off + NT],
                            start=(j == 0), stop=False,
                            tile_position=(0, 64))
                    for j in range(3, 6):    # taps (+1,dx), K=64
                        off = WP + (j - 4)
                        nc.tensor.matmul(
                            ps[0:64, :], lhsT=wg[0:64, j * F:(j + 1) * F],
                            rhs=xt[0:64, oA + off:oA + off + NT],
                            start=False, stop=(j == 5))
                        nc.tensor.matmul(
                            ps[64:128, :], lhsT=wg[0:64, j * F:(j + 1) * F],
                            rhs=xt[0:64, oB + off:oB + off + NT],
                            start=False, stop=(j == 5),
                            tile_position=(0, 64))
                    # drain + per-f bias, alternating ACT / DVE
                    dst = yt[:, t * NT:(t + 1) * NT]
                    if t % 2 == 0:
                        nc.scalar.activation(dst, ps[:], ACTF.Identity,
                                             bias=bagg_t[:, b:b + 1],
                                             scale=1.0)
                    else:
                        nc.vector.tensor_scalar(dst, ps[:],
                                                scalar1=bagg_t[:, b:b + 1],
                                                scalar2=None, op0=ALU.add)

                if dbg:
                    nc.sync.dma_start(dxt_d.ap()[b], xt[:, PADL:PADL + 512])
                    nc.sync.dma_start(dpool_d.ap()[b], pooled[:])
                    nc.sync.dma_start(dpib_d.ap()[b], pib[:])
                    nc.sync.dma_start(dwg_d.ap()[b], wg[:])

                # ---- store yT [f | spatial] straight to DRAM ----
                nc.gpsimd.dma_start(yp_d.ap()[b], yt[:])

    nc.compile()
    return nc


def _get_program():
    if "nc" not in _CACHE:
        _CACHE["nc"] = _build_program()
    return _CACHE["nc"]


def _host_pack_x(x):
    # [B, H, W, C] fp32 -> [B, 128, SP] bf16: rows 0:64 = width-padded x
    # transposed to [c, spatial], rows 64:128 = same, shifted one padded
    # image row down (row 64+c at col s holds x[c, s-WP]).
    xb = x.astype(BF16)
    xp = np.zeros((B, H, WP, C), dtype=BF16)
    xp[:, :, 1:W + 1, :] = xb
    flat = xp.reshape(B, SP, C)
    xT = np.ascontiguousarray(flat.transpose(0, 2, 1))   # [B, C, SP]
    x2 = np.zeros((B, 128, SP), dtype=BF16)
    x2[:, 0:C, :] = xT
    x2[:, C:2 * C, WP:] = xT[:, :, 0:SP - WP]
    return x2


def _host_pack_wk(Wk):
    # [K, 3, 3, C, F] -> [128, K*NSLOT*F] fp32. Slot j in 0..2 pairs taps
    # (kh=1, kw=j) on partitions 0:64 with (kh=0, kw=j) on 64:128 (the
    # bottom x half holds the row above); slot j in 3..5 holds (kh=2,
    # kw=j-3) on partitions 0:64, zeros on 64:128.
    w = np.zeros((128, KK, NSLOT, F), dtype=np.float32)
    wt = np.transpose(Wk, (3, 0, 1, 2, 4))          # [C, K, kh, kw, F]
    for j in range(3):
        w[0:C, :, j] = wt[:, :, 1, j]
        w[C:2 * C, :, j] = wt[:, :, 0, j]
        w[0:C, :, 3 + j] = wt[:, :, 2, j]
    return np.ascontiguousarray(w.reshape(128, KK * NSLOT * F))


def kernel(x, Wk, bk, att_w1, att_b1, att_w2, att_b2):
    from concourse import bass_utils

    nc = _get_program()

    x2 = _host_pack_x(np.asarray(x))
    wk_h = _host_pack_wk(np.asarray(Wk))
    w1_h = np.ascontiguousarray((att_w1 / (H * W)).astype(np.float32))
    b1_h = np.ascontiguousarray(att_b1.reshape(HID, 1).astype(np.float32))
    w2_h = np.ascontiguousarray((att_w2 / TEMP).astype(np.float32))
    b2_h = np.ascontiguousarray((att_b2 / TEMP).reshape(1, KK)
                                .astype(np.float32))
    bkt = np.transpose(bk, (1, 0)).astype(np.float32)      # [F, K]
    bkt_h = np.ascontiguousarray(np.concatenate([bkt, bkt], axis=0))

    in_maps = []
    for c in range(NCORES):
        in_maps.append({
            "x2": x2[c * BPC:(c + 1) * BPC],
            "wk": wk_h, "w1": w1_h, "b1": b1_h,
            "w2": w2_h, "b2": b2_h, "bkt": bkt_h,
        })

    res = bass_utils.run_bass_kernel_spmd(nc, in_maps,
                                          core_ids=list(range(NCORES)))

    y = np.empty((B, H, W, F), dtype=np.float32)
    for c in range(NCORES):
        yp = res.results[c]["ypad"]                 # [BPC, 128, HALF]
        arr = yp.reshape(BPC, 2, F, H // 2, WP)     # (b, half, f, row, col)
        y[c * BPC:(c + 1) * BPC] = (
            arr[:, :, :, :, 1:W + 1]
            .transpose(0, 1, 3, 4, 2)
            .reshape(BPC, H, W, F)
            .astype(np.float32))
    return y


# revision 4
# speedup vs baseline: 20.7774x; 4.8041x over previous
"""DynamicConv2D Trainium2 kernel (8-core SPMD, data-parallel over batch).

Per sample: GAP -> MLP -> softmax routing over K=4 kernel banks, weight-space
aggregation, then a 3x3 SAME conv with the per-sample aggregated kernel.

Device strategy (per core, 4 samples, fully per-sample pipelined):
  - Host packs x into a TRANSPOSED width-padded, channel-duplicated bf16
    layout [128, SP] per sample (rows 0:64 = channels at spatial s, rows
    64:128 = same data shifted one padded image row down, so row 64+c at
    position s holds x[c, s-WP]). One plain contiguous DMA per sample
    loads it -- no xbar-transpose DMAs anywhere (they serialized the SP
    engine in the previous design).
  - Pooled mean via one DVE free-dim reduction over xt rows 0:64.
  - Tiny routing MLP on PE (fp32) + softmax (DVE/ACT) -> pi [1, 4].
  - pi broadcast to all partitions (gpsimd), kernel bank aggregated on DVE
    with scalar_tensor_tensor FMA chains -> per-sample W_agg bf16.
  - Conv as shifted matmuls accumulating in PSUM: out[f, p] tiles, f on
    partitions. SBUF partitions 64:128 hold x shifted one padded image row
    up (the row above), so one K=128 matmul computes taps (dy=0, dx) and
    (dy=-1, dx) at once; the dy=+1 taps are K=64 matmuls at row base 0.
    Col groups (0/64) of the PE array run the two half-image tiles A/B
    concurrently.
  - PSUM drains (+per-f bias) alternate between ACT and DVE into bf16 yT
    [f-partitions, spatial]; one plain DMA stores yT per sample; host
    un-transposes and strips width pads, upcasts to fp32.
"""

import numpy as np
import ml_dtypes

BF16 = ml_dtypes.bfloat16

B, H, W, C, F = 32, 128, 128, 64, 64
KK, HID = 4, 16
TEMP = 30.0
NCORES, BPC = 8, 4
WP = W + 2          # padded width (zero col at w'=0 and w'=129)
SP = H * WP         # 16640 padded spatial per sample
PADL = 384          # SBUF zero halo before the image
PADR = 384          # SBUF zero halo after (taps read up to +2*WP+1 = 261)
NT = 416            # matmul moving-dim tile (PSUM bank: <=512 fp32)
HALF = SP // 2      # 8320, image halves A (h<64) / B (h>=64)
TPH = HALF // NT    # 20 tiles per half
NSLOT = 6           # 3 paired-tap slots (K=128) + 3 single-tap slots (K=64)

_CACHE = {}


def _build_program(dbg=False, reps=1):
    import concourse.bacc as bacc
    import concourse.mybir as mybir
    import concourse.tile as tile

    f32 = mybir.dt.float32
    bf16 = mybir.dt.bfloat16
    AX = mybir.AxisListType.X
    ALU = mybir.AluOpType
    ACTF = mybir.ActivationFunctionType

    nc = bacc.Bacc("TRN2", target_bir_lowering=False, debug=False)

    x2_d = nc.dram_tensor("x2", [BPC, 128, SP], bf16, kind="ExternalInput")
    wk_d = nc.dram_tensor("wk", [128, KK * NSLOT * F], f32,
                          kind="ExternalInput")
    w1_d = nc.dram_tensor("w1", [C, HID], f32, kind="ExternalInput")
    b1_d = nc.dram_tensor("b1", [HID, 1], f32, kind="ExternalInput")
    w2_d = nc.dram_tensor("w2", [HID, KK], f32, kind="ExternalInput")
    b2_d = nc.dram_tensor("b2", [1, KK], f32, kind="ExternalInput")
    bkt_d = nc.dram_tensor("bkt", [128, KK], f32, kind="ExternalInput")
    yp_d = nc.dram_tensor("ypad", [BPC, 128, HALF], bf16,
                          kind="ExternalOutput")
    if dbg:
        dxt_d = nc.dram_tensor("dxt", [BPC, 128, 512], bf16,
                               kind="ExternalOutput")
        dpool_d = nc.dram_tensor("dpool", [BPC, C, 1], f32,
                                 kind="ExternalOutput")
        dpib_d = nc.dram_tensor("dpib", [BPC, 128, KK], f32,
                                kind="ExternalOutput")
        dwg_d = nc.dram_tensor("dwg", [BPC, 128, NSLOT * F], bf16,
                               kind="ExternalOutput")

    with tile.TileContext(nc) as tc:
        from contextlib import ExitStack
        with ExitStack() as ctx:
            cst = ctx.enter_context(tc.tile_pool(name="cst", bufs=1))
            xtp = ctx.enter_context(tc.tile_pool(name="xtp", bufs=3))
            ytp = ctx.enter_context(tc.tile_pool(name="ytp", bufs=2))
            wgp = ctx.enter_context(tc.tile_pool(name="wgp", bufs=2))
            smp = ctx.enter_context(tc.tile_pool(name="smp", bufs=2))
            psp = ctx.enter_context(tc.tile_pool(name="psp", bufs=6, space="PSUM"))
            psr = ctx.enter_context(tc.tile_pool(name="psr", bufs=1, space="PSUM"))

            # ---- constants ----
            wk_t = cst.tile([128, KK * NSLOT * F], f32)
            nc.sync.dma_start(wk_t[:], wk_d.ap())
            w1_t = cst.tile([C, HID], f32)
            nc.sync.dma_start(w1_t[:], w1_d.ap())
            b1_t = cst.tile([HID, 1], f32)
            nc.sync.dma_start(b1_t[:], b1_d.ap())
            w2_t = cst.tile([HID, KK], f32)
            nc.sync.dma_start(w2_t[:], w2_d.ap())
            b2_t = cst.tile([1, KK], f32)
            nc.sync.dma_start(b2_t[:], b2_d.ap())
            bkt_t = cst.tile([128, KK], f32)
            nc.sync.dma_start(bkt_t[:], bkt_d.ap())
            bagg_t = cst.tile([128, BPC], f32)

            for _rep in range(reps):
              for b in range(BPC):
                # ---- load x (plain chunked DMA, already transposed on host)
                # and fold each chunk 4:1 into pooled partials with an
                # accumulate-DMA (SWDGE CCE add) so the big GAP reduction
                # never touches DVE/ACT.
                NCH, CHW, FW = 4, SP // 4, SP // 16   # 4 chunks, fold 4:1
                xt = xtp.tile([128, PADL + SP + PADR], bf16, tag="xt")
                nc.gpsimd.memset(xt[:, 0:PADL], 0.0)
                nc.gpsimd.memset(xt[:, PADL + SP:PADL + SP + PADR], 0.0)
                pacc = smp.tile([C, NCH, FW], bf16, tag="pacc")
                nc.gpsimd.memset(pacc[:], 0.0)
                for i in range(NCH):
                    o = i * CHW
                    nc.sync.dma_start(xt[:, PADL + o:PADL + o + CHW],
                                      x2_d.ap()[b][:, o:o + CHW])
                    nc.gpsimd.dma_start(
                        pacc[0:C, i:i + 1, :].broadcast_to([C, 4, FW]),
                        xt[0:C, PADL + o:PADL + o + CHW]
                        .rearrange("c (a w) -> c a w", a=4),
                        accum_op=ALU.add)

                # ---- pooled sum: tiny fold of the DMA partials ----
                pooled = smp.tile([C, 1], f32, tag="pooled")
                nc.vector.reduce_sum(pooled[:],
                                     pacc[:].rearrange("c a w -> c (a w)"),
                                     axis=AX)

                # ---- routing MLP (fp32, tiny) ----
                hps = psr.tile([HID, 1], f32, tag="hps")
                nc.tensor.matmul(hps[:], lhsT=w1_t[:], rhs=pooled[:],
                                 start=True, stop=True)
                h_t = smp.tile([HID, 1], f32, tag="h")
                nc.scalar.activation(h_t[:], hps[:], ACTF.Relu,
                                     bias=b1_t[:], scale=1.0)
                lps = psr.tile([1, KK], f32, tag="lps")
                nc.tensor.matmul(lps[:], lhsT=h_t[:], rhs=w2_t[:],
                                 start=True, stop=True)
                lg = smp.tile([1, KK], f32, tag="lg")
                nc.vector.tensor_tensor(lg[:], lps[:], b2_t[:], op=ALU.add)
                mx = smp.tile([1, 1], f32, tag="mx")
                nc.vector.reduce_max(mx[:], lg[:], axis=AX)
                ex = smp.tile([1, KK], f32, tag="ex")
                nc.vector.tensor_scalar(ex[:], lg[:], scalar1=mx[:],
                                        scalar2=None, op0=ALU.subtract)
                nc.scalar.activation(ex[:], ex[:], ACTF.Exp)
                sm = smp.tile([1, 1], f32, tag="sm")
                nc.vector.reduce_sum(sm[:], ex[:], axis=AX)
                rc = smp.tile([1, 1], f32, tag="rc")
                nc.vector.reciprocal(rc[:], sm[:])
                pi_t = smp.tile([1, KK], f32, tag="pi")
                nc.vector.tensor_scalar(pi_t[:], ex[:], scalar1=rc[:],
                                        scalar2=None, op0=ALU.mult)
                pib = smp.tile([128, KK], f32, tag="pib")
                nc.gpsimd.partition_broadcast(pib[:], pi_t[:])

                # ---- per-sample bias column: bagg[:, b] = sum_k bkT[:,k]*pi_k
                nc.vector.tensor_scalar(bagg_t[:, b:b + 1], bkt_t[:, 0:1],
                                        scalar1=pib[:, 0:1], scalar2=None,
                                        op0=ALU.mult)
                for k in range(1, KK):
                    nc.vector.scalar_tensor_tensor(
                        bagg_t[:, b:b + 1], bkt_t[:, k:k + 1],
                        pib[:, k:k + 1], bagg_t[:, b:b + 1],
                        op0=ALU.mult, op1=ALU.add)

                # ---- aggregate kernel bank: W_agg = sum_k pi_k * Wk ----
                SF = NSLOT * F
                acc = wgp.tile([128, SF], f32, tag="acc")
                nc.vector.tensor_scalar(acc[:], wk_t[:, 0:SF],
                                        scalar1=pib[:, 0:1], scalar2=None,
                                        op0=ALU.mult)
                for k in range(1, KK):
                    nc.vector.scalar_tensor_tensor(
                        acc[:], wk_t[:, k * SF:(k + 1) * SF],
                        pib[:, k:k + 1], acc[:], op0=ALU.mult, op1=ALU.add)
                wg = wgp.tile([128, SF], bf16, tag="wg")
                nc.vector.tensor_copy(wg[:], acc[:])

                # ---- conv: paired-tap K=128 + single-tap K=64 matmuls ----
                yt = ytp.tile([128, HALF], bf16, tag="yt")
                for t in range(TPH):
                    ps = psp.tile([128, NT], f32, tag="ps")
                    oA = PADL + t * NT
                    oB = oA + HALF
                    for j in range(3):       # taps (0,dx)+(-1,dx), K=128
                        off = j - 1
                        nc.tensor.matmul(
                            ps[0:64, :], lhsT=wg[:, j * F:(j + 1) * F],
                            rhs=xt[:, oA + off:oA + off + NT],
                            start=(j == 0), stop=False)
                        nc.tensor.matmul(
                            ps[64:128, :], lhsT=wg[:, j * F:(j + 1) * F],
                            rhs=xt[:, oB + off:oB + off + NT],
                            start=(j == 0), stop=False,
                            tile_position=(0, 64))
                    for j in range(3, 6):    # taps (+1,dx), K=64
                        off = WP + (j - 4)
                        nc.tensor.matmul(
                            ps[0:64, :], lhsT=wg[0:64, j * F:(j + 1) * F],
                            rhs=xt[0:64, oA + off:oA + off + NT],
                            start=False, stop=(j == 5))
                        nc.tensor.matmul(
                            ps[64:128, :], lhsT=wg[0:64, j * F:(j + 1) * F],
                            rhs=xt[0:64, oB + off:oB + off + NT],
                            start=False, stop=(j == 5),
                            tile_position=(0, 64))
                    # drain + per-f bias, alternating ACT / DVE
                    dst = yt[:, t * NT:(t + 1) * NT]
                    if t % 2 == 0:
                        nc.scalar.activation(dst, ps[:], ACTF.Identity,
                                             bias=bagg_t[:, b:b + 1],
                                             scale=1.0)
                    else:
                        nc.vector.tensor_scalar(dst, ps[:],
                                                scalar1=bagg_t[:, b:b + 1],
                                                scalar2=None, op0=ALU.add)

                if dbg:
                    nc.sync.dma_start(dxt_d.ap()[b], xt[:, PADL:PADL + 512])
                    nc.sync.dma_start(dpool_d.ap()[b], pooled[:])
                    nc.sync.dma_start(dpib_d.ap()[b], pib[:])
                    nc.sync.dma_start(dwg_d.ap()[b], wg[:])

                # ---- store yT [f | spatial] straight to DRAM ----
                # (scalar HWDGE queue: keeps gpsimd free for the next
                # sample's accumulate-DMAs)
                nc.scalar.dma_start(yp_d.ap()[b], yt[:])

    nc.compile()
    return nc


def _get_program():
    if "nc" not in _CACHE:
        _CACHE["nc"] = _build_program()
    return _CACHE["nc"]


def _host_pack_x(x):
    # [B, H, W, C] fp32 -> [B, 128, SP] bf16: rows 0:64 = width-padded x
    # transposed to [c, spatial], rows 64:128 = same, shifted one padded
    # image row down (row 64+c at col s holds x[c, s-WP]).
    xb = x.astype(BF16)
    xp = np.zeros((B, H, WP, C), dtype=BF16)
    xp[:, :, 1:W + 1, :] = xb
    flat = xp.reshape(B, SP, C)
    xT = np.ascontiguousarray(flat.transpose(0, 2, 1))   # [B, C, SP]
    x2 = np.zeros((B, 128, SP), dtype=BF16)
    x2[:, 0:C, :] = xT
    x2[:, C:2 * C, WP:] = xT[:, :, 0:SP - WP]
    return x2


def _host_pack_wk(Wk):
    # [K, 3, 3, C, F] -> [128, K*NSLOT*F] fp32. Slot j in 0..2 pairs taps
    # (kh=1, kw=j) on partitions 0:64 with (kh=0, kw=j) on 64:128 (the
    # bottom x half holds the row above); slot j in 3..5 holds (kh=2,
    # kw=j-3) on partitions 0:64, zeros on 64:128.
    w = np.zeros((128, KK, NSLOT, F), dtype=np.float32)
    wt = np.transpose(Wk, (3, 0, 1, 2, 4))          # [C, K, kh, kw, F]
    for j in range(3):
        w[0:C, :, j] = wt[:, :, 1, j]
        w[C:2 * C, :, j] = wt[:, :, 0, j]
        w[0:C, :, 3 + j] = wt[:, :, 2, j]
    return np.ascontiguousarray(w.reshape(128, KK * NSLOT * F))


def kernel(x, Wk, bk, att_w1, att_b1, att_w2, att_b2):
    from concourse import bass_utils

    nc = _get_program()

    x2 = _host_pack_x(np.asarray(x))
    wk_h = _host_pack_wk(np.asarray(Wk))
    w1_h = np.ascontiguousarray((att_w1 / (H * W)).astype(np.float32))
    b1_h = np.ascontiguousarray(att_b1.reshape(HID, 1).astype(np.float32))
    w2_h = np.ascontiguousarray((att_w2 / TEMP).astype(np.float32))
    b2_h = np.ascontiguousarray((att_b2 / TEMP).reshape(1, KK)
                                .astype(np.float32))
    bkt = np.transpose(bk, (1, 0)).astype(np.float32)      # [F, K]
    bkt_h = np.ascontiguousarray(np.concatenate([bkt, bkt], axis=0))

    in_maps = []
    for c in range(NCORES):
        in_maps.append({
            "x2": x2[c * BPC:(c + 1) * BPC],
            "wk": wk_h, "w1": w1_h, "b1": b1_h,
            "w2": w2_h, "b2": b2_h, "bkt": bkt_h,
        })

    res = bass_utils.run_bass_kernel_spmd(nc, in_maps,
                                          core_ids=list(range(NCORES)))

    y = np.empty((B, H, W, F), dtype=np.float32)
    for c in range(NCORES):
        yp = res.results[c]["ypad"]                 # [BPC, 128, HALF]
        arr = yp.reshape(BPC, 2, F, H // 2, WP)     # (b, half, f, row, col)
        y[c * BPC:(c + 1) * BPC] = (
            arr[:, :, :, :, 1:W + 1]
            .transpose(0, 1, 3, 4, 2)
            .reshape(BPC, H, W, F)
            .astype(np.float32))
    return y
